# revision 12
# baseline (speedup 1.0000x reference)
"""AFTSimple (attention-free transformer, simple variant) distributed Trainium2 kernel.

Reference math (B=1, S=8192, E=1024, all f32):
    Q = q @ Wq.T + bq                     # [S, E]
    K = q @ Wk.T + bk                     # [S, E]
    V = q @ Wv.T + bv                     # [S, E]
    w = softmax(K, axis=S)                # per-feature softmax over sequence
    c = sum_f sum_s w[s,f] * V[s,f]       # scalar
    Y = sigmoid(Q) * c                    # [S, E]

Distribution: shard S across 8 NeuronCores (1024 rows each), replicate weights.
Per-core stats (sum_s exp(K), sum_s exp(K)*Vraw) are AllReduced (8 KiB), the
bv contribution is applied after the collective:
    numer_f = AR(sum_s exp(K)*Vraw)_f + bv_f * AR(sum_s exp(K))_f
No max-subtraction in the softmax: K values are O(1) for this problem
(|K| < ~6), exp() is safe in f32.

Compute dtype: bf16 matmuls with f32 PSUM accumulation; everything after the
projections is f32.
"""

import os
import sys

for _p in ("/opt/trn_rl_repo", "/root/.axon_site/_ro/trn_rl_repo"):
    if os.path.isdir(_p) and _p not in sys.path:
        sys.path.insert(0, _p)

import numpy as np

B, S, E = 1, 8192, 1024
N_CORES = 8
P = 128
S_SH = S // N_CORES          # 1024 rows of q per core
EC = E // P                  # 8 contraction chunks
FC = E // P                  # 8 output-feature chunks
SC = S_SH // P               # 8 sequence chunks per core
NHALF = 512                  # PSUM bank: 512 f32 per partition

_CACHE = {}


def _build_nc(use_collective=True):
    import concourse.bass as bass
    import concourse.bacc as bacc
    import concourse.tile as tile
    from concourse import mybir
    from concourse.masks import make_identity

    f32 = mybir.dt.float32
    bf16 = mybir.dt.bfloat16
    AF = mybir.ActivationFunctionType

    nc = bacc.Bacc("TRN2", target_bir_lowering=False, debug=False,
                   num_devices=N_CORES)

    q_ext = nc.dram_tensor("q", [S_SH, E], f32, kind="ExternalInput")
    Wq_ext = nc.dram_tensor("Wq", [E, E], f32, kind="ExternalInput")
    bq_ext = nc.dram_tensor("bq", [E], f32, kind="ExternalInput")
    Wk_ext = nc.dram_tensor("Wk", [E, E], f32, kind="ExternalInput")
    bk_ext = nc.dram_tensor("bk", [E], f32, kind="ExternalInput")
    Wv_ext = nc.dram_tensor("Wv", [E, E], f32, kind="ExternalInput")
    bv_ext = nc.dram_tensor("bv", [E], f32, kind="ExternalInput")
    out_ext = nc.dram_tensor("out", [S_SH, E], f32, kind="ExternalOutput")

    # Collective bounce buffers (collectives can't touch kernel I/O tensors).
    stats_in = nc.dram_tensor("stats_in", [P, 16], f32)
    stats_out = nc.dram_tensor("stats_out", [P, 16], f32, addr_space="Shared")
    c_dram = nc.dram_tensor("c_scalar", [1], f32)

    from contextlib import ExitStack
    with tile.TileContext(nc) as tc, ExitStack() as ctx:
        const = ctx.enter_context(tc.tile_pool(name="const", bufs=1))
        stage = ctx.enter_context(tc.tile_pool(name="stage", bufs=5))
        persist = ctx.enter_context(tc.tile_pool(name="persist", bufs=1))
        epool = ctx.enter_context(tc.tile_pool(name="epool", bufs=2))
        small = ctx.enter_context(tc.tile_pool(name="small", bufs=1))
        ysigp = ctx.enter_context(tc.tile_pool(name="ysigp", bufs=1))
        tpsum = ctx.enter_context(tc.tile_pool(name="tpsum", bufs=2, space="PSUM"))
        kvpsum = ctx.enter_context(tc.tile_pool(name="kvpsum", bufs=2, space="PSUM"))
        qpsum = ctx.enter_context(tc.tile_pool(name="qpsum", bufs=2, space="PSUM"))

        # ---- constants -------------------------------------------------
        ident = const.tile([P, P], bf16)
        make_identity(nc, ident)
        ones1 = const.tile([1, P], bf16)
        nc.vector.memset(ones1, 1.0)

        # biases: bk gathered as [128, 8] (partition p holds f = c*128+p),
        # bv flat on one partition in (p, c) order for the c-chain,
        # bq as a bf16 row [1, E] for the K=1 bias matmul.
        bk_sb = const.tile([P, FC], f32)
        nc.gpsimd.dma_start(out=bk_sb, in_=bk_ext.ap().rearrange("(c p) -> p c", p=P))
        bv_flat = const.tile([1, P, FC], f32)
        nc.gpsimd.dma_start(out=bv_flat,
                            in_=bv_ext.ap().rearrange("(o c p) -> o p c", o=1, p=P))
        bq_bf = const.tile([1, E], bf16)
        nc.gpsimd.dma_start(out=bq_bf, in_=bq_ext.ap().rearrange("(o e) -> o e", o=1))

        stats = small.tile([P, 16], f32)   # cols 0..7 numer, 8..15 denom

        # ---- on-chip transpose helper ---------------------------------
        # src_dram is [rows=1024, E]; produce dstT[e_chunk] = [128e, 1024rows]
        # bf16 tiles.  f32->bf16 cast happens in the (SWDGE) DMA.
        def load_transposed(src_dram, dst_name):
            dstT = [persist.tile([P, S_SH], bf16, tag=f"{dst_name}{e}",
                                 name=f"{dst_name}{e}")
                    for e in range(EC)]
            for half in range(2):           # row chunks 0..3 / 4..7
                stg = []
                for j in range(4):
                    r0 = (half * 4 + j) * P
                    t = stage.tile([P, E], bf16, tag="stg")
                    nc.gpsimd.dma_start(out=t, in_=src_dram[r0:r0 + P, :])
                    stg.append(t)
                for e in range(EC):
                    tp = tpsum.tile([P, NHALF], bf16, tag="tp")
                    for j in range(4):
                        nc.tensor.transpose(
                            tp[:, j * P:(j + 1) * P],
                            stg[j][:, e * P:(e + 1) * P],
                            ident,
                        )
                    nc.any.tensor_copy(
                        out=dstT[e][:, half * NHALF:(half + 1) * NHALF],
                        in_=tp,
                    )
            return dstT

        WkT = load_transposed(Wk_ext, "WkT")
        WvT = load_transposed(Wv_ext, "WvT")
        qT = load_transposed(q_ext, "qT")

        # ---- K / V projections + softmax stats (layout [f, s]) ---------
        for f in range(FC):
            fsl = slice(f * P, (f + 1) * P)
            kk = kvpsum.tile([P, S_SH], f32, tag="kv")
            for e in range(EC):
                for h in range(2):
                    nc.tensor.matmul(
                        kk[:, h * NHALF:(h + 1) * NHALF],
                        lhsT=WkT[e][:, fsl],
                        rhs=qT[e][:, h * NHALF:(h + 1) * NHALF],
                        start=(e == 0), stop=(e == EC - 1),
                    )
            et = epool.tile([P, S_SH], f32, tag="et")
            nc.scalar.activation(
                out=et, in_=kk, func=AF.Exp,
                bias=bk_sb[:, f:f + 1], scale=1.0,
                accum_out=stats[:, 8 + f:9 + f],
            )

            vv = kvpsum.tile([P, S_SH], f32, tag="kv")
            for e in range(EC):
                for h in range(2):
                    nc.tensor.matmul(
                        vv[:, h * NHALF:(h + 1) * NHALF],
                        lhsT=WvT[e][:, fsl],
                        rhs=qT[e][:, h * NHALF:(h + 1) * NHALF],
                        start=(e == 0), stop=(e == EC - 1),
                    )
            prod = epool.tile([P, S_SH], f32, tag="prod")
            nc.vector.tensor_mul(prod, et, vv)
            nc.vector.reduce_sum(stats[:, f:f + 1], prod,
                                 axis=mybir.AxisListType.X)

        # ---- AllReduce of the 8 KiB stats ------------------------------
        nc.gpsimd.dma_start(out=stats_in[:, :], in_=stats)
        if use_collective:
            nc.gpsimd.collective_compute(
                "AllReduce",
                mybir.AluOpType.add,
                replica_groups=[list(range(N_CORES))],
                ins=[stats_in.ap().opt()],
                outs=[stats_out.ap().opt()],
            )
        else:
            nc.gpsimd.dma_start(out=stats_out[:, :], in_=stats_in[:, :])

        # ---- Q projection + sigmoid (layout [s, f]); overlaps collective
        WqT = load_transposed(Wq_ext, "WqT")
        ysig = []
        for s in range(SC):
            ssl = slice(s * P, (s + 1) * P)
            ys = ysigp.tile([P, E], f32, tag=f"ysig{s}")
            for h in range(2):
                qp = qpsum.tile([P, NHALF], f32, tag="qp")
                for e in range(EC):
                    nc.tensor.matmul(
                        qp,
                        lhsT=qT[e][:, ssl],
                        rhs=WqT[e][:, h * NHALF:(h + 1) * NHALF],
                        start=(e == 0), stop=False,
                    )
                # += bq (rank-1 update with a ones column)
                nc.tensor.matmul(
                    qp,
                    lhsT=ones1,
                    rhs=bq_bf[:, h * NHALF:(h + 1) * NHALF],
                    start=False, stop=True,
                )
                nc.scalar.activation(
                    out=ys[:, h * NHALF:(h + 1) * NHALF], in_=qp,
                    func=AF.Sigmoid,
                )
            ysig.append(ys)

        # ---- global context scalar c (off the PE: one-partition DVE chain)
        # numer_f = AR_numer_raw + bv_f * AR_denom ; c = sum_f numer_f/denom_f
        sflat = small.tile([1, P, 16], f32)
        nc.gpsimd.dma_start(out=sflat,
                            in_=stats_out.ap().rearrange("(o p) c -> o p c", o=1))
        nm = small.tile([1, P, FC], f32)
        nc.vector.tensor_mul(nm, bv_flat, sflat[:, :, 8:16])
        nc.vector.tensor_add(nm, nm, sflat[:, :, 0:8])
        rec = small.tile([1, P, FC], f32)
        nc.vector.reciprocal(rec, sflat[:, :, 8:16])
        nc.vector.tensor_mul(nm, nm, rec)
        csc = small.tile([1, 1], f32)
        nc.vector.reduce_sum(csc, nm, axis=mybir.AxisListType.XY)
        nc.gpsimd.dma_start(out=c_dram.ap().rearrange("(o c) -> o c", o=1), in_=csc)
        c_sb = small.tile([P, 1], f32)
        c_bcast = bass.AP(tensor=c_dram.ap().tensor, offset=0, ap=[[0, P], [1, 1]])
        nc.gpsimd.dma_start(out=c_sb, in_=c_bcast)

        # ---- Y = sigmoid(Q) * c, stream out ----------------------------
        for s in range(SC):
            nc.vector.tensor_scalar_mul(ysig[s], ysig[s], c_sb)
            nc.scalar.dma_start(out=out_ext[s * P:(s + 1) * P, :], in_=ysig[s])

    nc.compile()
    return nc


def _get_nc():
    if "nc" not in _CACHE:
        _CACHE["nc"] = _build_nc()
    return _CACHE["nc"]


def _make_in_maps(q, Wq, bq, Wk, bk, Wv, bv):
    q = np.ascontiguousarray(np.asarray(q, dtype=np.float32).reshape(S, E))
    Wq = np.ascontiguousarray(np.asarray(Wq, dtype=np.float32))
    Wk = np.ascontiguousarray(np.asarray(Wk, dtype=np.float32))
    Wv = np.ascontiguousarray(np.asarray(Wv, dtype=np.float32))
    bq = np.ascontiguousarray(np.asarray(bq, dtype=np.float32))
    bk = np.ascontiguousarray(np.asarray(bk, dtype=np.float32))
    bv = np.ascontiguousarray(np.asarray(bv, dtype=np.float32))
    in_maps = []
    for i in range(N_CORES):
        in_maps.append({
            "q": q[i * S_SH:(i + 1) * S_SH],
            "Wq": Wq, "bq": bq, "Wk": Wk, "bk": bk, "Wv": Wv, "bv": bv,
        })
    return in_maps


def _run(trace=False, **inputs):
    from concourse.bass_utils import run_bass_kernel_spmd
    nc = _get_nc()
    in_maps = _make_in_maps(**inputs)
    res = run_bass_kernel_spmd(nc, in_maps, core_ids=list(range(N_CORES)),
                               trace=trace)
    shards = [np.asarray(res.results[i]["out"]) for i in range(N_CORES)]
    out = np.concatenate(shards, axis=0).reshape(B, S, E).astype(np.float32)
    return out, res


def kernel(**inputs):
    out, _ = _run(trace=False, **inputs)
    return out
